# revision 31
# baseline (speedup 1.0000x reference)
"""Depthwise 4x4 blur (upfirdn2d pad=(2,1)) on TRN2, 8 NeuronCores.

Math: out[h,w] = sum_{i,j} Kf[i,j] * x[h+i-2, w+j-2]   (Kf = flipped 2D kernel,
out-of-range terms = zero padding). For each kernel column j this is a banded
128x128 matrix A_j applied over H to a W-shifted slice of the padded image:

    OUT = sum_j A_j @ Xpad[:, j:j+128]      (PSUM accumulation over j)

H-padding is folded into the band clipping of A_j; W-padding is baked into the
host-side layout (stride-131 rows: [0, 0, x0..x127, 0]). Sharding: batch dim
(8 batches -> 8 cores), each core processes 256 images of 128x128.

bf16 everywhere (tolerance is 2e-2; measured rel err ~4e-3): halves HBM
traffic vs f32 (HBM floor ~47 us/core) and streams the PE at 1 col/cycle.
Host-side the block is transposed to [H, C, WP] / [H, C, W] so every DMA is a
dense 2D pattern (multi-KB contiguous run per partition).

v3: the blur is separable, so A_j = kw[j] * Ab. Taps pair up:

    kw0*Ab@Xp0 + kw1*Ab@Xp1 = A_0 @ (Xp0 + (kw1/kw0)*Xp1) = A_0 @ u
    kw3*Ab@Xp3 + kw2*Ab@Xp2 = A_3 @ (Xp3 + (kw2/kw3)*Xp2) = A_3 @ v

u/v are ONE DVE op each (tensor_tensor add for the symmetric kernel,
scalar_tensor_tensor otherwise), so prep-path groups need 2 matmuls instead
of 4, shifting PE work to the Vector engine. Measured pitfalls baked into the
layout below:
  - GpSimd shares SBUF ports with the DVE: running prep on both throttles
    both to ~1.18us/op, so GpSimd is excluded from the rotation.
  - The PE's HAM clock gate idles at 1.2 GHz; dummy warmup matmuls on a
    GpSimd-memset scratch run during the (framework-fixed ~11.5us) startup
    so real matmuls stream at 2.4 GHz from the first group.
  - Prep ops are emitted before the matmul loop, and prep groups sit last
    in PE program order, so the DVE never stalls the PE head-of-line.
  - PSUM->SBUF bf16 copies ride ACT (~0.64us/group); DVE takes the first
    groups of each big supertile so ACT's queue keeps up with PSUM bank
    reuse (8 banks, 16 groups in flight per 64-image supertile).
Final: ~66us/core vs ~150us for the f32 hilo baseline (DMA floor ~47us,
framework entry ~11.5us, drain ~5us).
"""

import numpy as np
from contextlib import ExitStack

import concourse.bass as bass
import concourse.bacc as bacc
import concourse.tile as tile
import concourse.mybir as mybir
from concourse.bass_utils import run_bass_kernel_spmd

N_CORES = 8
B, C, H, W = 8, 256, 128, 128
WP = W + 3         # padded image stride: [0, 0, x0..x127, 0]
GROUP = 4          # images per PSUM bank (4*128 = 512 f32 = one bank)
SUPER = 64         # images per DMA (~16 KB contiguous per partition)
MM4_EVERY = 4      # every Nth group uses the 4-matmul path (PE/DVE balance)

F32 = mybir.dt.float32
BF16 = mybir.dt.bfloat16
MULT = mybir.AluOpType.mult
ADD = mybir.AluOpType.add


def _body(ctx, tc, o_ap, x_ap, w_ap, s_u, s_v):
    nc = tc.nc
    wpool = ctx.enter_context(tc.tile_pool(name="wts", bufs=1))
    xpool = ctx.enter_context(tc.tile_pool(name="xin", bufs=4))
    upool = ctx.enter_context(tc.tile_pool(name="uv", bufs=12))
    opool = ctx.enter_context(tc.tile_pool(name="oup", bufs=4))
    ppool = ctx.enter_context(tc.tile_pool(name="ps", bufs=8, space="PSUM"))

    wt = wpool.tile([H, 4 * H], BF16)

    # ramp-up / ramp-down supertile sizes: small tiles at the ends prime and
    # drain the DMA->prep->matmul->copy->DMA pipeline faster
    sizes = [4, 8, 16, 32] + [SUPER] * 2 + [32, 16, 8, 8, 4]
    assert sum(sizes) == C

    # PE warmup: the HAM clock gate needs ~3.4us of sustained activity to
    # release 2.4 GHz. GpSimd comes out of the preamble earliest (~6us), so
    # a GPS-memset scratch lets dummy matmuls run while the first data DMA
    # is still in flight -- real matmuls then start (and stay) warm.
    scratch = wpool.tile([H, 4 * W], BF16)
    nc.gpsimd.memset(scratch[:], 0)
    warm_pt = ppool.tile([H, GROUP * W], F32, tag="pt")
    for _ in range(9):
        nc.tensor.matmul(warm_pt[:], scratch[:, :H], scratch[:], start=True, stop=True)

    def emit_prep(peng, xt3, uv3, gi, gc):
        # GpSimd can't touch PSUM and only has plain tensor_tensor, but the
        # SBUF->SBUF prep adds are exactly that (when the kernel is symmetric)
        for k, (ja, jb, s) in enumerate(((0, 3, s_u), (1, 2, s_v))):
            va = xt3[:, gi : gi + gc, ja : ja + W]
            vb = xt3[:, gi : gi + gc, jb : jb + W]
            if s == 1.0:
                peng.tensor_tensor(uv3[:, k], va, vb, ADD)
            else:
                nc.vector.scalar_tensor_tensor(uv3[:, k], vb, s, va, MULT, ADD)

    n_sup = len(sizes)
    offs = [sum(sizes[:i]) for i in range(n_sup)]

    def super_groups(sz):
        return [(i * GROUP, min(GROUP, sz - i * GROUP))
                for i in range((sz + GROUP - 1) // GROUP)]

    # per-supertile engine pattern for the 8-group supers:
    #   positions 0-2 mm4-path (PE), 3-5 DVE-prep, 6-7 GPS-prep.
    # prep ops are emitted BEFORE the mm loop (GPS even one supertile early),
    # so the slower vector engines always stay ahead of the PE's program
    # order and never stall it head-of-line.
    # NOTE: GpSimd shares its SBUF read/write ports with the DVE — running
    # prep adds on both throttles BOTH to ~1.18us/op (measured). GpSimd
    # therefore stays out of the steady-state rotation entirely.
    def path_of(ng, gidx):
        if ng == 16:
            return "dve" if gidx >= 5 else "mm4"
        if ng == 8:
            return "dve" if gidx >= 4 else "mm4"
        if ng == 4:
            return "dve" if gidx == 3 else "mm4"
        return "mm4"

    xt3s = {}

    def load_super(s):
        # input DMAs all ride the SP ring: the ACT ring's preamble
        # (ACT_TABLE_LOAD) lands ~8.5us and would gate the first matmul.
        # Big supertiles arrive as two half-DMAs with independent
        # completion semaphores, so the PE can start on the first half
        # ~2us before the full tile lands.
        if s < n_sup and s not in xt3s:
            sz = sizes[s]
            xt = xpool.tile([H, sz * WP], BF16, tag="xt")
            xt3 = xt[:].rearrange("h (c w) -> h c w", c=sz)
            if sz >= 32:
                hf = sz // 2
                nc.sync.dma_start(xt3[:, :hf], x_ap[:, offs[s] : offs[s] + hf])
                nc.sync.dma_start(xt3[:, hf:], x_ap[:, offs[s] + hf : offs[s] + sz])
            else:
                nc.sync.dma_start(xt3, x_ap[:, offs[s] : offs[s] + sz])
            xt3s[s] = xt3

    gps_uv = {}

    def emit_gps(s):
        # GPS is slow (~1.2us/add): give it a full supertile of lead time
        if s >= n_sup or s in gps_uv:
            return
        gps_uv[s] = {}
        groups = super_groups(sizes[s])
        for gidx, (gi, gc) in enumerate(groups):
            if path_of(len(groups), gidx) == "gps":
                uv = upool.tile([H, 2 * gc * W], BF16, tag="uv")
                uv3 = uv[:].rearrange("h (k c w) -> h k c w", k=2, c=gc)
                emit_prep(nc.gpsimd, xt3s[s], uv3, gi, gc)
                gps_uv[s][gidx] = uv3

    load_super(0)
    nc.sync.dma_start(wt[:], w_ap)
    emit_gps(0)
    for s, sz in enumerate(sizes):
        load_super(s + 1)
        emit_gps(s + 1)
        xt3 = xt3s.pop(s)
        groups = super_groups(sz)
        ng = len(groups)
        prep = gps_uv.pop(s)
        for gidx, (gi, gc) in enumerate(groups):
            if path_of(ng, gidx) == "dve":
                uv = upool.tile([H, 2 * gc * W], BF16, tag="uv")
                uv3 = uv[:].rearrange("h (k c w) -> h k c w", k=2, c=gc)
                emit_prep(nc.vector, xt3, uv3, gi, gc)
                prep[gidx] = uv3
        ot = opool.tile([H, sz * W], BF16, tag="ot")
        for gidx, (gi, gc) in enumerate(groups):
            pt = ppool.tile([H, gc * W], F32, tag="pt")
            if gidx in prep:
                uv3 = prep[gidx]
                nc.tensor.matmul(pt[:], wt[:, :H], uv3[:, 0], start=True, stop=False)
                nc.tensor.matmul(pt[:], wt[:, H : 2 * H], uv3[:, 1], start=False, stop=True)
            else:
                for j in range(4):
                    nc.tensor.matmul(
                        pt[:], wt[:, j * H : (j + 1) * H],
                        xt3[:, gi : gi + gc, j : j + W],
                        start=(j == 0), stop=(j == 3),
                    )
            od = ot[:, gi * W : (gi + gc) * W]
            # copies mostly on ACT; DVE takes the first groups of big
            # supers, whose matmuls finish while DVE is still ahead
            if (ng == 16 and gidx in (0, 1)) or (ng == 8 and gidx == 0):
                nc.vector.tensor_copy(od, pt[:])
            else:
                nc.scalar.copy(od, pt[:])
            # output DMA per pair of groups (8 images, 2 KB/partition
            # descriptors): output flows during compute instead of bunching
            # into a multi-us drain after the last matmul
            if gidx % 2 == 1 or gidx == ng - 1:
                p0 = groups[gidx - 1][0] if gidx % 2 == 1 else gi
                pc = gi + gc - p0
                oeng = nc.sync if (s >= n_sup - 3 and gidx % 4 == 1) else nc.scalar
                oeng.dma_start(
                    o_ap[:, offs[s] + p0 : offs[s] + p0 + pc],
                    ot[:, p0 * W : (p0 + pc) * W].rearrange(
                        "h (c w) -> h c w", c=pc
                    ),
                )


def build_module(s_u, s_v):
    nc = bacc.Bacc(
        "TRN2", target_bir_lowering=False, debug=False, num_devices=N_CORES
    )
    x_ap = nc.dram_tensor("x", [H, C, WP], BF16, kind="ExternalInput").ap()
    w_ap = nc.dram_tensor("wts", [H, 4 * H], BF16, kind="ExternalInput").ap()
    o_ap = nc.dram_tensor("out", [H, C, W], BF16, kind="ExternalOutput").ap()
    with tile.TileContext(nc) as tc:
        with ExitStack() as ctx:
            _body(ctx, tc, o_ap, x_ap, w_ap, s_u, s_v)
    nc.compile()
    return nc


def band_mats(k2d):
    """WT[j] = A_j^T where A_j[h, h+i-2] = Kf[i, j] (rows clipped to [0,128))."""
    kf = np.asarray(k2d, np.float32)[::-1, ::-1]
    wts = np.zeros((4, H, H), np.float32)
    for j in range(4):
        for i in range(4):
            d = i - 2  # diagonal offset m - h
            h0, h1 = max(0, -d), min(H, H - d)
            idx = np.arange(h0, h1)
            wts[j, idx + d, idx] = kf[i, j]
    return wts


def _bf16(a):
    import ml_dtypes

    return np.asarray(a).astype(ml_dtypes.bfloat16)


def prep_x(x_core):
    """[C,H,W] f32 -> [H,C,WP] bf16 with zero cols at 0,1 and WP-1."""
    xp = np.zeros((H, x_core.shape[0], WP), np.float32)
    xp[:, :, 2 : 2 + W] = x_core.transpose(1, 0, 2)
    return _bf16(xp)


_module_cache = {}


def kernel(x, kernel, _trace=False, _trace_kwargs=None):
    x = np.asarray(x, np.float32)
    assert x.shape == (B, C, H, W), x.shape
    kf = np.asarray(kernel, np.float32)[::-1, ::-1]
    # column sums of the flipped 2D kernel = flipped 1D W-kernel (separable)
    kw = kf.sum(axis=0) / kf.sum()
    s_u = float(kw[3] / kw[0])  # u = Xp0 + s_u*Xp3 under stationary A_0
    s_v = float(kw[2] / kw[1])  # v = Xp1 + s_v*Xp2 under stationary A_1
    key = (round(s_u, 9), round(s_v, 9))
    if key not in _module_cache:
        _module_cache[key] = build_module(s_u, s_v)
    nc = _module_cache[key]
    wts = _bf16(band_mats(kernel).transpose(1, 0, 2).reshape(H, 4 * H))
    in_maps = [{"x": prep_x(x[i]), "wts": wts.copy()} for i in range(N_CORES)]
    res = run_bass_kernel_spmd(
        nc, in_maps, list(range(N_CORES)), trace=_trace, **(_trace_kwargs or {})
    )
    out = np.stack(
        [
            np.asarray(res.results[i]["out"]).transpose(1, 0, 2).astype(np.float32)
            for i in range(N_CORES)
        ],
        axis=0,
    )
    if _trace:
        return out, res
    return out


# revision 33
# speedup vs baseline: 1.0188x; 1.0188x over previous
"""Depthwise 4x4 blur (upfirdn2d pad=(2,1)) on TRN2, 8 NeuronCores.

Math: out[h,w] = sum_{i,j} Kf[i,j] * x[h+i-2, w+j-2]   (Kf = flipped 2D kernel,
out-of-range terms = zero padding). For each kernel column j this is a banded
128x128 matrix A_j applied over H to a W-shifted slice of the padded image:

    OUT = sum_j A_j @ Xpad[:, j:j+128]      (PSUM accumulation over j)

H-padding is folded into the band clipping of A_j; W-padding is baked into the
host-side layout (stride-131 rows: [0, 0, x0..x127, 0]). Sharding: batch dim
(8 batches -> 8 cores), each core processes 256 images of 128x128.

bf16 everywhere (tolerance is 2e-2; measured rel err ~4e-3): halves HBM
traffic vs f32 (HBM floor ~47 us/core) and streams the PE at 1 col/cycle.
Host-side the block is transposed to [H, C, WP] / [H, C, W] so every DMA is a
dense 2D pattern (multi-KB contiguous run per partition).

v3: the blur is separable, so A_j = kw[j] * Ab. Taps pair up:

    kw0*Ab@Xp0 + kw1*Ab@Xp1 = A_0 @ (Xp0 + (kw1/kw0)*Xp1) = A_0 @ u
    kw3*Ab@Xp3 + kw2*Ab@Xp2 = A_3 @ (Xp3 + (kw2/kw3)*Xp2) = A_3 @ v

u/v are ONE DVE op each (tensor_tensor add for the symmetric kernel,
scalar_tensor_tensor otherwise), so prep-path groups need 2 matmuls instead
of 4, shifting PE work to the Vector engine. Measured pitfalls baked into the
layout below:
  - GpSimd shares SBUF ports with the DVE: running prep on both throttles
    both to ~1.18us/op, so GpSimd is excluded from the rotation.
  - The PE's HAM clock gate idles at 1.2 GHz; dummy warmup matmuls on a
    GpSimd-memset scratch run during the (framework-fixed ~11.5us) startup
    so real matmuls stream at 2.4 GHz from the first group.
  - Prep ops are emitted before the matmul loop, and prep groups sit last
    in PE program order, so the DVE never stalls the PE head-of-line.
  - PSUM->SBUF bf16 copies ride ACT (~0.64us/group); DVE takes the first
    groups of each big supertile so ACT's queue keeps up with PSUM bank
    reuse (8 banks, 16 groups in flight per 64-image supertile).
Final: ~66us/core vs ~150us for the f32 hilo baseline (DMA floor ~47us,
framework entry ~11.5us, drain ~5us).
"""

import numpy as np
from contextlib import ExitStack

import concourse.bass as bass
import concourse.bacc as bacc
import concourse.tile as tile
import concourse.mybir as mybir
from concourse.bass_utils import run_bass_kernel_spmd

N_CORES = 8
B, C, H, W = 8, 256, 128, 128
WP = W + 3         # padded image stride: [0, 0, x0..x127, 0]
GROUP = 4          # images per PSUM bank (4*128 = 512 f32 = one bank)
SUPER = 64         # images per DMA (~16 KB contiguous per partition)
MM4_EVERY = 4      # every Nth group uses the 4-matmul path (PE/DVE balance)

F32 = mybir.dt.float32
BF16 = mybir.dt.bfloat16
MULT = mybir.AluOpType.mult
ADD = mybir.AluOpType.add


def _body(ctx, tc, o_ap, x_ap, w_ap, s_u, s_v):
    nc = tc.nc
    wpool = ctx.enter_context(tc.tile_pool(name="wts", bufs=1))
    xpool = ctx.enter_context(tc.tile_pool(name="xin", bufs=4))
    upool = ctx.enter_context(tc.tile_pool(name="uv", bufs=12))
    opool = ctx.enter_context(tc.tile_pool(name="oup", bufs=4))
    ppool = ctx.enter_context(tc.tile_pool(name="ps", bufs=8, space="PSUM"))

    wt = wpool.tile([H, 4 * H], BF16)

    # ramp-up / ramp-down supertile sizes: small tiles at the ends prime and
    # drain the DMA->prep->matmul->copy->DMA pipeline faster
    sizes = [4, 8, 16, 32] + [SUPER] * 2 + [32, 16, 8, 8, 4]
    assert sum(sizes) == C

    # PE warmup: the HAM clock gate needs ~3.4us of sustained activity to
    # release 2.4 GHz. GpSimd comes out of the preamble earliest (~6us), so
    # a GPS-memset scratch lets dummy matmuls run while the first data DMA
    # is still in flight -- real matmuls then start (and stay) warm.
    scratch = wpool.tile([H, 4 * W], BF16)
    nc.gpsimd.memset(scratch[:], 0)
    warm_pt = ppool.tile([H, GROUP * W], F32, tag="pt")
    for _ in range(9):
        nc.tensor.matmul(warm_pt[:], scratch[:, :H], scratch[:], start=True, stop=True)

    def emit_prep(peng, xt3, uv3, gi, gc):
        # GpSimd can't touch PSUM and only has plain tensor_tensor, but the
        # SBUF->SBUF prep adds are exactly that (when the kernel is symmetric)
        for k, (ja, jb, s) in enumerate(((0, 3, s_u), (1, 2, s_v))):
            va = xt3[:, gi : gi + gc, ja : ja + W]
            vb = xt3[:, gi : gi + gc, jb : jb + W]
            if s == 1.0:
                peng.tensor_tensor(uv3[:, k], va, vb, ADD)
            else:
                nc.vector.scalar_tensor_tensor(uv3[:, k], vb, s, va, MULT, ADD)

    n_sup = len(sizes)
    offs = [sum(sizes[:i]) for i in range(n_sup)]

    def super_groups(sz):
        return [(i * GROUP, min(GROUP, sz - i * GROUP))
                for i in range((sz + GROUP - 1) // GROUP)]

    # per-supertile engine pattern for the 8-group supers:
    #   positions 0-2 mm4-path (PE), 3-5 DVE-prep, 6-7 GPS-prep.
    # prep ops are emitted BEFORE the mm loop (GPS even one supertile early),
    # so the slower vector engines always stay ahead of the PE's program
    # order and never stall it head-of-line.
    # NOTE: GpSimd shares its SBUF read/write ports with the DVE — running
    # prep adds on both throttles BOTH to ~1.18us/op (measured). GpSimd
    # therefore stays out of the steady-state rotation entirely.
    def path_of(ng, gidx):
        if ng == 16:
            return "dve" if gidx >= 6 else "mm4"
        if ng == 8:
            return "dve" if gidx >= 5 else "mm4"
        if ng == 4:
            return "dve" if gidx == 3 else "mm4"
        return "mm4"

    xt3s = {}

    def load_super(s):
        # input DMAs all ride the SP ring: the ACT ring's preamble
        # (ACT_TABLE_LOAD) lands ~8.5us and would gate the first matmul.
        # Big supertiles arrive as two half-DMAs with independent
        # completion semaphores, so the PE can start on the first half
        # ~2us before the full tile lands.
        if s < n_sup and s not in xt3s:
            sz = sizes[s]
            xt = xpool.tile([H, sz * WP], BF16, tag="xt")
            xt3 = xt[:].rearrange("h (c w) -> h c w", c=sz)
            if sz >= 32:
                hf = sz // 2
                nc.sync.dma_start(xt3[:, :hf], x_ap[:, offs[s] : offs[s] + hf])
                nc.sync.dma_start(xt3[:, hf:], x_ap[:, offs[s] + hf : offs[s] + sz])
            else:
                nc.sync.dma_start(xt3, x_ap[:, offs[s] : offs[s] + sz])
            xt3s[s] = xt3

    gps_uv = {}

    def emit_gps(s):
        # GPS is slow (~1.2us/add): give it a full supertile of lead time
        if s >= n_sup or s in gps_uv:
            return
        gps_uv[s] = {}
        groups = super_groups(sizes[s])
        for gidx, (gi, gc) in enumerate(groups):
            if path_of(len(groups), gidx) == "gps":
                uv = upool.tile([H, 2 * gc * W], BF16, tag="uv")
                uv3 = uv[:].rearrange("h (k c w) -> h k c w", k=2, c=gc)
                emit_prep(nc.gpsimd, xt3s[s], uv3, gi, gc)
                gps_uv[s][gidx] = uv3

    load_super(0)
    nc.sync.dma_start(wt[:], w_ap)
    emit_gps(0)
    for s, sz in enumerate(sizes):
        load_super(s + 1)
        emit_gps(s + 1)
        xt3 = xt3s.pop(s)
        groups = super_groups(sz)
        ng = len(groups)
        prep = gps_uv.pop(s)
        for gidx, (gi, gc) in enumerate(groups):
            if path_of(ng, gidx) == "dve":
                uv = upool.tile([H, 2 * gc * W], BF16, tag="uv")
                uv3 = uv[:].rearrange("h (k c w) -> h k c w", k=2, c=gc)
                emit_prep(nc.vector, xt3, uv3, gi, gc)
                prep[gidx] = uv3
        ot = opool.tile([H, sz * W], BF16, tag="ot")
        for gidx, (gi, gc) in enumerate(groups):
            pt = ppool.tile([H, gc * W], F32, tag="pt")
            if gidx in prep:
                uv3 = prep[gidx]
                nc.tensor.matmul(pt[:], wt[:, :H], uv3[:, 0], start=True, stop=False)
                nc.tensor.matmul(pt[:], wt[:, H : 2 * H], uv3[:, 1], start=False, stop=True)
            else:
                for j in range(4):
                    nc.tensor.matmul(
                        pt[:], wt[:, j * H : (j + 1) * H],
                        xt3[:, gi : gi + gc, j : j + W],
                        start=(j == 0), stop=(j == 3),
                    )
            od = ot[:, gi * W : (gi + gc) * W]
            # copies mostly on ACT; DVE takes the first groups of big
            # supers, whose matmuls finish while DVE is still ahead
            if (ng == 16 and gidx in (0, 1)) or (ng == 8 and gidx == 0):
                nc.vector.tensor_copy(od, pt[:])
            else:
                nc.scalar.copy(od, pt[:])
            # output DMA per quarter supertile (16 images): output flows
            # during compute instead of bunching into a multi-us drain after
            # the last matmul. Not finer: the Tile scheduler has only 8 DMA
            # completion lanes, and >6-7 DMAs in flight per super serializes.
            qn = 4 if ng == 16 else (2 if ng == 8 else 1)
            per = ng // qn
            if (gidx + 1) % per == 0:
                p0 = groups[gidx + 1 - per][0]
                pc = gi + gc - p0
                oeng = nc.sync if (s >= n_sup - 2 and gidx % 2) else nc.scalar
                oeng.dma_start(
                    o_ap[:, offs[s] + p0 : offs[s] + p0 + pc],
                    ot[:, p0 * W : (p0 + pc) * W].rearrange(
                        "h (c w) -> h c w", c=pc
                    ),
                )


def build_module(s_u, s_v):
    nc = bacc.Bacc(
        "TRN2", target_bir_lowering=False, debug=False, num_devices=N_CORES
    )
    x_ap = nc.dram_tensor("x", [H, C, WP], BF16, kind="ExternalInput").ap()
    w_ap = nc.dram_tensor("wts", [H, 4 * H], BF16, kind="ExternalInput").ap()
    o_ap = nc.dram_tensor("out", [H, C, W], BF16, kind="ExternalOutput").ap()
    with tile.TileContext(nc) as tc:
        with ExitStack() as ctx:
            _body(ctx, tc, o_ap, x_ap, w_ap, s_u, s_v)
    nc.compile()
    return nc


def band_mats(k2d):
    """WT[j] = A_j^T where A_j[h, h+i-2] = Kf[i, j] (rows clipped to [0,128))."""
    kf = np.asarray(k2d, np.float32)[::-1, ::-1]
    wts = np.zeros((4, H, H), np.float32)
    for j in range(4):
        for i in range(4):
            d = i - 2  # diagonal offset m - h
            h0, h1 = max(0, -d), min(H, H - d)
            idx = np.arange(h0, h1)
            wts[j, idx + d, idx] = kf[i, j]
    return wts


def _bf16(a):
    import ml_dtypes

    return np.asarray(a).astype(ml_dtypes.bfloat16)


def prep_x(x_core):
    """[C,H,W] f32 -> [H,C,WP] bf16 with zero cols at 0,1 and WP-1."""
    xp = np.zeros((H, x_core.shape[0], WP), np.float32)
    xp[:, :, 2 : 2 + W] = x_core.transpose(1, 0, 2)
    return _bf16(xp)


_module_cache = {}


def kernel(x, kernel, _trace=False, _trace_kwargs=None):
    x = np.asarray(x, np.float32)
    assert x.shape == (B, C, H, W), x.shape
    kf = np.asarray(kernel, np.float32)[::-1, ::-1]
    # column sums of the flipped 2D kernel = flipped 1D W-kernel (separable)
    kw = kf.sum(axis=0) / kf.sum()
    s_u = float(kw[3] / kw[0])  # u = Xp0 + s_u*Xp3 under stationary A_0
    s_v = float(kw[2] / kw[1])  # v = Xp1 + s_v*Xp2 under stationary A_1
    key = (round(s_u, 9), round(s_v, 9))
    if key not in _module_cache:
        _module_cache[key] = build_module(s_u, s_v)
    nc = _module_cache[key]
    wts = _bf16(band_mats(kernel).transpose(1, 0, 2).reshape(H, 4 * H))
    in_maps = [{"x": prep_x(x[i]), "wts": wts.copy()} for i in range(N_CORES)]
    res = run_bass_kernel_spmd(
        nc, in_maps, list(range(N_CORES)), trace=_trace, **(_trace_kwargs or {})
    )
    out = np.stack(
        [
            np.asarray(res.results[i]["out"]).transpose(1, 0, 2).astype(np.float32)
            for i in range(N_CORES)
        ],
        axis=0,
    )
    if _trace:
        return out, res
    return out


# revision 35
# speedup vs baseline: 1.1475x; 1.1263x over previous
"""Depthwise 4x4 blur (upfirdn2d pad=(2,1)) on TRN2, 8 NeuronCores.

Math: out[h,w] = sum_{i,j} Kf[i,j] * x[h+i-2, w+j-2]   (Kf = flipped 2D kernel,
out-of-range terms = zero padding). For each kernel column j this is a banded
128x128 matrix A_j applied over H to a W-shifted slice of the padded image:

    OUT = sum_j A_j @ Xpad[:, j:j+128]      (PSUM accumulation over j)

H-padding is folded into the band clipping of A_j; W-padding is baked into the
host-side layout (stride-131 rows: [0, 0, x0..x127, 0]). Sharding: batch dim
(8 batches -> 8 cores), each core processes 256 images of 128x128.

bf16 everywhere (tolerance is 2e-2; measured rel err ~4e-3): halves HBM
traffic vs f32 (HBM floor ~47 us/core) and streams the PE at 1 col/cycle.
Host-side the block is transposed to [H, C, WP] / [H, C, W] so every DMA is a
dense 2D pattern (multi-KB contiguous run per partition).

v3: the blur is separable, so A_j = kw[j] * Ab. Taps pair up:

    kw0*Ab@Xp0 + kw1*Ab@Xp1 = A_0 @ (Xp0 + (kw1/kw0)*Xp1) = A_0 @ u
    kw3*Ab@Xp3 + kw2*Ab@Xp2 = A_3 @ (Xp3 + (kw2/kw3)*Xp2) = A_3 @ v

u/v are ONE DVE op each (tensor_tensor add for the symmetric kernel,
scalar_tensor_tensor otherwise), so prep-path groups need 2 matmuls instead
of 4, shifting PE work to the Vector engine. Measured pitfalls baked into the
layout below:
  - GpSimd shares SBUF ports with the DVE: running prep on both throttles
    both to ~1.18us/op, so GpSimd is excluded from the rotation.
  - The PE's HAM clock gate idles at 1.2 GHz; dummy warmup matmuls on a
    GpSimd-memset scratch run during the (framework-fixed ~11.5us) startup
    so real matmuls stream at 2.4 GHz from the first group.
  - Prep ops are emitted before the matmul loop, and prep groups sit last
    in PE program order, so the DVE never stalls the PE head-of-line.
  - PSUM->SBUF bf16 copies ride ACT (~0.64us/group); DVE takes the first
    groups of each big supertile so ACT's queue keeps up with PSUM bank
    reuse (8 banks, 16 groups in flight per 64-image supertile).
Final: ~66us/core vs ~150us for the f32 hilo baseline (DMA floor ~47us,
framework entry ~11.5us, drain ~5us).
"""

import numpy as np
from contextlib import ExitStack

import concourse.bass as bass
import concourse.bacc as bacc
import concourse.tile as tile
import concourse.mybir as mybir
from concourse.bass_utils import run_bass_kernel_spmd

N_CORES = 8
B, C, H, W = 8, 256, 128, 128
WP = W + 3         # padded image stride: [0, 0, x0..x127, 0]
GROUP = 4          # images per PSUM bank (4*128 = 512 f32 = one bank)
SUPER = 64         # images per DMA (~16 KB contiguous per partition)
MM4_EVERY = 4      # every Nth group uses the 4-matmul path (PE/DVE balance)

F32 = mybir.dt.float32
BF16 = mybir.dt.bfloat16
MULT = mybir.AluOpType.mult
ADD = mybir.AluOpType.add


def _body(ctx, tc, o_ap, x_ap, w_ap, s_u, s_v):
    nc = tc.nc
    wpool = ctx.enter_context(tc.tile_pool(name="wts", bufs=1))
    xpool = ctx.enter_context(tc.tile_pool(name="xin", bufs=4))
    upool = ctx.enter_context(tc.tile_pool(name="uv", bufs=12))
    opool = ctx.enter_context(tc.tile_pool(name="oup", bufs=4))
    ppool = ctx.enter_context(tc.tile_pool(name="ps", bufs=8, space="PSUM"))

    wt = wpool.tile([H, 4 * H], BF16)

    # ramp-up / ramp-down supertile sizes: small tiles at the ends prime and
    # drain the DMA->prep->matmul->copy->DMA pipeline faster
    sizes = [4, 8, 16, 32] + [SUPER] * 2 + [32, 16, 8, 8, 4]
    assert sum(sizes) == C

    # PE warmup: the HAM clock gate needs ~3.4us of sustained activity to
    # release 2.4 GHz. GpSimd comes out of the preamble earliest (~6us), so
    # a GPS-memset scratch lets dummy matmuls run while the first data DMA
    # is still in flight -- real matmuls then start (and stay) warm.
    scratch = wpool.tile([H, 4 * W], BF16)
    nc.gpsimd.memset(scratch[:], 0)
    warm_pt = ppool.tile([H, GROUP * W], F32, tag="pt")
    for _ in range(9):
        nc.tensor.matmul(warm_pt[:], scratch[:, :H], scratch[:], start=True, stop=True)

    def emit_prep(peng, xt3, uv3, gi, gc):
        # GpSimd can't touch PSUM and only has plain tensor_tensor, but the
        # SBUF->SBUF prep adds are exactly that (when the kernel is symmetric)
        for k, (ja, jb, s) in enumerate(((0, 3, s_u), (1, 2, s_v))):
            va = xt3[:, gi : gi + gc, ja : ja + W]
            vb = xt3[:, gi : gi + gc, jb : jb + W]
            if s == 1.0:
                peng.tensor_tensor(uv3[:, k], va, vb, ADD)
            else:
                nc.vector.scalar_tensor_tensor(uv3[:, k], vb, s, va, MULT, ADD)

    n_sup = len(sizes)
    offs = [sum(sizes[:i]) for i in range(n_sup)]

    def super_groups(sz):
        return [(i * GROUP, min(GROUP, sz - i * GROUP))
                for i in range((sz + GROUP - 1) // GROUP)]

    # per-supertile engine pattern for the 8-group supers:
    #   positions 0-2 mm4-path (PE), 3-5 DVE-prep, 6-7 GPS-prep.
    # prep ops are emitted BEFORE the mm loop (GPS even one supertile early),
    # so the slower vector engines always stay ahead of the PE's program
    # order and never stall it head-of-line.
    # NOTE: GpSimd shares its SBUF read/write ports with the DVE — running
    # prep adds on both throttles BOTH to ~1.18us/op (measured). GpSimd
    # therefore stays out of the steady-state rotation entirely.
    def path_of(ng, gidx):
        if ng == 16:
            return "dve" if gidx >= 5 else "mm4"
        if ng == 8:
            return "dve" if gidx >= 4 else "mm4"
        if ng == 4:
            return "dve" if gidx == 3 else "mm4"
        return "mm4"

    xt3s = {}

    def load_super(s):
        # input DMAs all ride the SP ring: the ACT ring's preamble
        # (ACT_TABLE_LOAD) lands ~8.5us and would gate the first matmul.
        # Big supertiles arrive as two half-DMAs with independent
        # completion semaphores, so the PE can start on the first half
        # ~2us before the full tile lands.
        if s < n_sup and s not in xt3s:
            sz = sizes[s]
            xt = xpool.tile([H, sz * WP], BF16, tag="xt")
            xt3 = xt[:].rearrange("h (c w) -> h c w", c=sz)
            if sz >= 32:
                hf = sz // 2
                nc.sync.dma_start(xt3[:, :hf], x_ap[:, offs[s] : offs[s] + hf])
                nc.sync.dma_start(xt3[:, hf:], x_ap[:, offs[s] + hf : offs[s] + sz])
            else:
                nc.sync.dma_start(xt3, x_ap[:, offs[s] : offs[s] + sz])
            xt3s[s] = xt3

    gps_uv = {}

    def emit_gps(s):
        # GPS is slow (~1.2us/add): give it a full supertile of lead time
        if s >= n_sup or s in gps_uv:
            return
        gps_uv[s] = {}
        groups = super_groups(sizes[s])
        for gidx, (gi, gc) in enumerate(groups):
            if path_of(len(groups), gidx) == "gps":
                uv = upool.tile([H, 2 * gc * W], BF16, tag="uv")
                uv3 = uv[:].rearrange("h (k c w) -> h k c w", k=2, c=gc)
                emit_prep(nc.gpsimd, xt3s[s], uv3, gi, gc)
                gps_uv[s][gidx] = uv3

    load_super(0)
    nc.sync.dma_start(wt[:], w_ap)
    emit_gps(0)
    for s, sz in enumerate(sizes):
        load_super(s + 1)
        emit_gps(s + 1)
        xt3 = xt3s.pop(s)
        groups = super_groups(sz)
        ng = len(groups)
        prep = gps_uv.pop(s)
        for gidx, (gi, gc) in enumerate(groups):
            if path_of(ng, gidx) == "dve":
                uv = upool.tile([H, 2 * gc * W], BF16, tag="uv")
                uv3 = uv[:].rearrange("h (k c w) -> h k c w", k=2, c=gc)
                emit_prep(nc.vector, xt3, uv3, gi, gc)
                prep[gidx] = uv3
        ot = opool.tile([H, sz * W], BF16, tag="ot")
        for gidx, (gi, gc) in enumerate(groups):
            pt = ppool.tile([H, gc * W], F32, tag="pt")
            if gidx in prep:
                uv3 = prep[gidx]
                nc.tensor.matmul(pt[:], wt[:, :H], uv3[:, 0], start=True, stop=False)
                nc.tensor.matmul(pt[:], wt[:, H : 2 * H], uv3[:, 1], start=False, stop=True)
            else:
                for j in range(4):
                    nc.tensor.matmul(
                        pt[:], wt[:, j * H : (j + 1) * H],
                        xt3[:, gi : gi + gc, j : j + W],
                        start=(j == 0), stop=(j == 3),
                    )
            od = ot[:, gi * W : (gi + gc) * W]
            # copies mostly on ACT; DVE takes the first groups of big
            # supers, whose matmuls finish while DVE is still ahead
            if (ng == 16 and gidx in (0, 1)) or (ng == 8 and gidx == 0):
                nc.vector.tensor_copy(od, pt[:])
            else:
                nc.scalar.copy(od, pt[:])
            # half-supertile output DMAs on alternating rings: finer split
            # was tried (quarters, pairs) and LOSES — too many in-flight
            # DMAs overflow the Tile scheduler's 8 completion lanes
            if gidx == ng // 2 - 1 and ng >= 4:
                half = groups[ng // 2 - 1]
                hc = half[0] + half[1]
                nc.scalar.dma_start(
                    o_ap[:, offs[s] : offs[s] + hc],
                    ot[:, : hc * W].rearrange("h (c w) -> h c w", c=hc),
                )
        hc0 = 0 if ng < 4 else groups[ng // 2 - 1][0] + groups[ng // 2 - 1][1]
        nc.sync.dma_start(
            o_ap[:, offs[s] + hc0 : offs[s] + sz],
            ot[:, hc0 * W :].rearrange("h (c w) -> h c w", c=sz - hc0),
        )


def build_module(s_u, s_v):
    nc = bacc.Bacc(
        "TRN2", target_bir_lowering=False, debug=False, num_devices=N_CORES
    )
    x_ap = nc.dram_tensor("x", [H, C, WP], BF16, kind="ExternalInput").ap()
    w_ap = nc.dram_tensor("wts", [H, 4 * H], BF16, kind="ExternalInput").ap()
    o_ap = nc.dram_tensor("out", [H, C, W], BF16, kind="ExternalOutput").ap()
    with tile.TileContext(nc) as tc:
        with ExitStack() as ctx:
            _body(ctx, tc, o_ap, x_ap, w_ap, s_u, s_v)
    nc.compile()
    return nc


def band_mats(k2d):
    """WT[j] = A_j^T where A_j[h, h+i-2] = Kf[i, j] (rows clipped to [0,128))."""
    kf = np.asarray(k2d, np.float32)[::-1, ::-1]
    wts = np.zeros((4, H, H), np.float32)
    for j in range(4):
        for i in range(4):
            d = i - 2  # diagonal offset m - h
            h0, h1 = max(0, -d), min(H, H - d)
            idx = np.arange(h0, h1)
            wts[j, idx + d, idx] = kf[i, j]
    return wts


def _bf16(a):
    import ml_dtypes

    return np.asarray(a).astype(ml_dtypes.bfloat16)


def prep_x(x_core):
    """[C,H,W] f32 -> [H,C,WP] bf16 with zero cols at 0,1 and WP-1."""
    xp = np.zeros((H, x_core.shape[0], WP), np.float32)
    xp[:, :, 2 : 2 + W] = x_core.transpose(1, 0, 2)
    return _bf16(xp)


_module_cache = {}


def kernel(x, kernel, _trace=False, _trace_kwargs=None):
    x = np.asarray(x, np.float32)
    assert x.shape == (B, C, H, W), x.shape
    kf = np.asarray(kernel, np.float32)[::-1, ::-1]
    # column sums of the flipped 2D kernel = flipped 1D W-kernel (separable)
    kw = kf.sum(axis=0) / kf.sum()
    s_u = float(kw[3] / kw[0])  # u = Xp0 + s_u*Xp3 under stationary A_0
    s_v = float(kw[2] / kw[1])  # v = Xp1 + s_v*Xp2 under stationary A_1
    key = (round(s_u, 9), round(s_v, 9))
    if key not in _module_cache:
        _module_cache[key] = build_module(s_u, s_v)
    nc = _module_cache[key]
    wts = _bf16(band_mats(kernel).transpose(1, 0, 2).reshape(H, 4 * H))
    in_maps = [{"x": prep_x(x[i]), "wts": wts.copy()} for i in range(N_CORES)]
    res = run_bass_kernel_spmd(
        nc, in_maps, list(range(N_CORES)), trace=_trace, **(_trace_kwargs or {})
    )
    out = np.stack(
        [
            np.asarray(res.results[i]["out"]).transpose(1, 0, 2).astype(np.float32)
            for i in range(N_CORES)
        ],
        axis=0,
    )
    if _trace:
        return out, res
    return out
